# revision 28
# baseline (speedup 1.0000x reference)
"""Trainium2 Bass kernel for the per-pixel locally-connected MLP (dense_mlp).

Reference computation (per batch b, pixel (h,w)):
    x0 = coor (2-vector, shared by all pixels)
    h1 = relu(W0 @ x0)        W0 = weight[b, 0:32].reshape(16, 2)   per pixel
    h2 = relu(W1 @ h1)        W1 = weight[b, 32:288].reshape(16,16) per pixel
    y  = W2 @ h2 + bias       W2 = weight[b, 288:336].reshape(3,16), bias = weight[b,336]
Output: [4, 3, 256, 256] float32.

Sharding: 8 cores, core k handles batch k//2, image rows (k%2)*128:(k%2+1)*128
=> per-core weight shard [337, 32768] (channels x pixels); no cross-core comm.

Implementation notes:
- Channels live on SBUF partitions, pixels on the free axis, so every weight
  load is a wide contiguous DMA; weights are cast to fp16 on the host to halve
  HBM traffic (the kernel is purely memory-bound; rel err ~5e-4).
- The per-pixel matvecs are elementwise multiplies (VectorE) plus
  partition-axis reductions (TensorE matmuls against small host-built 0/1
  selection matrices; `coor` is folded into the first matmul's stationary
  matrix, the bias channel into the last one's moving operand). Matmul
  operands use float32r (TF32-like) for 4x PE throughput over fp32.
- Sub-chunks of 512 px (one PSUM bank) are processed in pairs that share PSUM
  banks: the pair's two halves land at partitions 0:16 / 32:48 of one bank via
  zero-padded stationaries accumulating at a base-0 dst, so each ScalarE
  relu / output-copy covers two chunks in one op.
- Work is emitted stage-major over 4096-px macro-tiles (3 merged HWDGE loads
  per macro + SWDGE bias/output DMAs on gpsimd) so the in-order engine queues
  pipeline across sub-chunks; pool buffer counts are sized to keep 2-3 macros
  in flight without deadlocking the Tile scheduler.
"""

import sys

for _p in ("/opt/trn_rl_repo", "/root/.axon_site/_ro/trn_rl_repo"):
    if _p not in sys.path:
        sys.path.append(_p)

import numpy as np

import concourse.bass as bass
import concourse.tile as tile
from concourse import bacc, mybir
from concourse.bass_utils import run_bass_kernel_spmd

# ---------------------------------------------------------------- constants
B, H, W = 4, 256, 256
N_CH = 337            # 32 (L0) + 256 (L1) + 48 (L2) + 1 (bias)
N_CORES = 8
PIX = (B * H * W) // N_CORES  # 32768 pixels per core
F = 512               # pixels per compute chunk (one PSUM bank of fp32)
N_CHUNKS = PIX // F

FP32 = mybir.dt.float32
FP32R = mybir.dt.float32r
FP16 = mybir.dt.float16


def _const_mats(coor: np.ndarray) -> dict[str, np.ndarray]:
    """Small stationary matrices for the TensorE reductions."""
    cx, cy = float(coor[0]), float(coor[1])
    # even/odd stationaries write one pair-half each at base-0 dst APs:
    # cols 0:16 = even chunk, cols 32:48 = odd chunk, zeros elsewhere
    # (zeros accumulate as no-ops into the other half's partitions).
    s0 = np.zeros((2, 32, 48), np.float32)    # h1pre = S0.T @ w[0:32]
    for h in range(2):
        for i in range(16):
            s0[h, 2 * i, 32 * h + i] = cx
            s0[h, 2 * i + 1, 32 * h + i] = cy
    r8 = np.zeros((16, 128), np.float32)      # h1rep[m] = h1[m % 16]
    for m in range(128):
        r8[m % 16, m] = 1.0
    m1a = np.zeros((2, 128, 48), np.float32)  # h2pre[j] += sum_i prodA[16j+i]
    m1b = np.zeros((2, 128, 48), np.float32)
    for h in range(2):
        for k in range(128):
            m1a[h, k, 32 * h + k // 16] = 1.0
            m1b[h, k, 32 * h + 8 + k // 16] = 1.0
    # pair-fused layer-2: moving operand is h2pair [48,F] with chunk A's h2
    # at rows 0:16 and chunk B's at rows 32:48 (rows 16:32 are junk)
    r3_2 = np.zeros((48, 96), np.float32)     # h2rep2[k] = h2(k//48)[k % 16]
    for k in range(96):
        r3_2[(0 if k < 48 else 32) + k % 16, k] = 1.0
    m2b2 = np.zeros((98, 6), np.float32)      # y[h*3+j] = sum prodC + bias
    for k in range(96):
        m2b2[k, (k // 48) * 3 + (k % 48) // 16] = 1.0
    m2b2[96, 0:3] = 1.0                       # bias row, even chunk
    m2b2[97, 3:6] = 1.0                       # bias row, odd chunk
    return {"s0": s0.astype(np.float16), "r8": r8, "m1a": m1a, "m1b": m1b,
            "r3_2": r3_2, "m2b2": m2b2}


def build_nc(repeat: int = 1):
    """Build the per-core Bass program. `repeat` re-runs the whole kernel
    body sequentially (used only for differential HW timing)."""
    nc = bacc.Bacc(None, target_bir_lowering=False)

    w = nc.declare_dram_parameter("w", [N_CH, PIX], FP16, isOutput=False)
    out = nc.declare_dram_parameter("out", [3, PIX], FP32, isOutput=True)
    c_s0 = nc.declare_dram_parameter("s0", [2, 32, 48], FP16, isOutput=False)
    c_r8 = nc.declare_dram_parameter("r8", [16, 128], FP32R, isOutput=False)
    c_m1a = nc.declare_dram_parameter("m1a", [2, 128, 48], FP32R, isOutput=False)
    c_m1b = nc.declare_dram_parameter("m1b", [2, 128, 48], FP32R, isOutput=False)
    c_r3_2 = nc.declare_dram_parameter("r3_2", [48, 96], FP32R, isOutput=False)
    c_m2b2 = nc.declare_dram_parameter("m2b2", [98, 6], FP32R, isOutput=False)

    G = 8                      # chunks per software-pipeline group
    with tile.TileContext(nc) as tc:
        with (
            tc.tile_pool(name="consts", bufs=1) as consts,
            tc.tile_pool(name="loads", bufs=2) as loads,
            tc.tile_pool(name="acts", bufs=4) as acts,
            tc.tile_pool(name="prods", bufs=3) as prods,
            tc.tile_pool(name="outs", bufs=2) as outs,
            tc.tile_pool(name="ps_sm16", bufs=3, space="PSUM") as ps_sm16,
            tc.tile_pool(name="ps_h2p", bufs=2, space="PSUM") as ps_h2p,
            tc.tile_pool(name="ps_rep", bufs=3, space="PSUM") as ps_rep,
        ):
            s0 = consts.tile([32, 2, 48], FP16)
            r8 = consts.tile([48, 128], FP32R)   # rows 0:16 and 32:48 both
                                                 # hold R8 (for base 0/32)
            m1a = consts.tile([128, 2, 48], FP32R)
            m1b = consts.tile([128, 2, 48], FP32R)
            r3_2 = consts.tile([48, 96], FP32R)
            m2b2 = consts.tile([98, 6], FP32R)
            for t, d in ((s0, c_s0.rearrange("h k m -> k h m")),
                         (r8[0:16, :], c_r8), (r8[32:48, :], c_r8),
                         (m1a, c_m1a.rearrange("h k m -> k h m")),
                         (m1b, c_m1b.rearrange("h k m -> k h m")),
                         (r3_2, c_r3_2), (m2b2, c_m2b2)):
                nc.sync.dma_start(out=t[:], in_=d[:])

            relu = mybir.ActivationFunctionType.Relu

            def body():
                # Macro-tile of G*F pixels; inside, sub-chunks are processed
                # in PAIRS sharing PSUM banks at partition offsets 0/32 (both
                # legal matmul base partitions). This halves ACT op count and
                # fuses all of layer 2 (rep, products, reduce+bias) per pair.
                FM = G * F
                NP = G // 2     # pairs per macro
                for g in range(N_CHUNKS // G):
                    mp = slice(g * FM, (g + 1) * FM)
                    sls = [slice(i * F, (i + 1) * F) for i in range(G)]
                    psl = [slice(p * F, (p + 1) * F) for p in range(NP)]

                    t0m = loads.tile([32, FM], FP16, tag="t0", name="t0m", bufs=3)
                    t1m = loads.tile([128, 2, FM], FP16, tag="t1", name="t1m", bufs=3)
                    # t2 pair-stacked: partition h*48+ch, free (pair, x);
                    # h = parity of the sub-chunk within its pair
                    t2m = loads.tile([96, NP, F], FP16, tag="t2", name="t2m", bufs=3)
                    pcm = prods.tile([98, NP, F], FP32R, tag="pcm", name="pcm",
                                     bufs=3)
                    nc.sync.dma_start(out=t0m[:], in_=w[0:32, mp])
                    nc.sync.dma_start(
                        out=t1m[:],
                        in_=w[32:288, mp].rearrange("(b p) x -> p b x", b=2))
                    for h in range(2):
                        nc.sync.dma_start(
                            out=t2m[48 * h:48 * h + 48, :, :],
                            in_=bass.AP(tensor=w[:].tensor,
                                        offset=288 * PIX + g * FM + h * F,
                                        ap=[[PIX, 48], [2 * F, NP], [1, F]]))
                        nc.gpsimd.dma_start(
                            out=pcm[96 + h:97 + h, :, :],
                            in_=bass.AP(tensor=w[:].tensor,
                                        offset=336 * PIX + g * FM + h * F,
                                        ap=[[2 * F, NP], [1, F]]))

                    h1pre = {}
                    for p in range(NP):
                        h1pre[p] = ps_sm16.tile([48, F], FP32, tag="sm16",
                                                name="h1pre")
                        nc.tensor.matmul(h1pre[p][:], s0[:, 0, :],
                                         t0m[:, sls[2 * p]],
                                         start=True, stop=False)
                        nc.tensor.matmul(h1pre[p][:], s0[:, 1, :],
                                         t0m[:, sls[2 * p + 1]],
                                         start=False, stop=True)
                    h1 = {}
                    for p in range(NP):
                        h1[p] = acts.tile([48, F], FP32R, tag="h1", name="h1")
                        nc.scalar.activation(h1[p][:], h1pre[p][:], relu)
                    h1rep = {}
                    for p in range(NP):
                        for h in range(2):
                            h1rep[p, h] = ps_rep.tile([128, F], FP32,
                                                      tag="rep", name="h1rep")
                            nc.tensor.matmul(
                                h1rep[p, h][:],
                                r8[32 * h:32 * h + 16, :],
                                h1[p][32 * h:32 * h + 16, :],
                                start=True, stop=True)
                    prodAB = {}
                    for p in range(NP):
                        for h in range(2):
                            prodAB[p, h] = prods.tile([128, 2, F], FP32R,
                                                      tag="prodAB",
                                                      name="prodAB", bufs=3)
                            rp = h1rep[p, h]
                            rep2 = bass.AP(tensor=rp.tensor,
                                           offset=rp[:].offset,
                                           ap=[rp[:].ap[0], [0, 2],
                                               rp[:].ap[1]])
                            nc.vector.tensor_mul(
                                prodAB[p, h][:],
                                t1m[:, :, sls[2 * p + h]], rep2)
                    h2pre = {}
                    for p in range(NP):
                        h2pre[p] = ps_h2p.tile([48, F], FP32, tag="h2p",
                                               name="h2pre")
                        for h in range(2):
                            nc.tensor.matmul(h2pre[p][:],
                                             m1a[:, h, :], prodAB[p, h][:, 0, :],
                                             start=(h == 0), stop=False)
                            nc.tensor.matmul(h2pre[p][:],
                                             m1b[:, h, :], prodAB[p, h][:, 1, :],
                                             start=False, stop=(h == 1))
                    h2 = {}
                    for p in range(NP):
                        h2[p] = acts.tile([48, F], FP32R, tag="h2", name="h2")
                        nc.scalar.activation(h2[p][:], h2pre[p][:], relu)
                    h2rep = {}
                    for p in range(NP):
                        h2rep[p] = ps_rep.tile([96, F], FP32, tag="rep",
                                               name="h2rep")
                        nc.tensor.matmul(h2rep[p][:], r3_2[:], h2[p][:],
                                         start=True, stop=True)
                    for p in range(NP):
                        nc.vector.tensor_mul(pcm[0:96, p, :],
                                             t2m[:, p, :], h2rep[p][:])
                    y = {}
                    for p in range(NP):
                        y[p] = ps_sm16.tile([6, F], FP32, tag="sm16", name="y")
                        nc.tensor.matmul(y[p][:], m2b2[:], pcm[:, p, :],
                                         start=True, stop=True)
                    y_sb = outs.tile([6, NP, F], FP32, tag="ysb", name="ysb",
                                     bufs=2)
                    for p in range(NP):
                        nc.scalar.copy(y_sb[:, p, :], y[p][:])
                    # y_sb partition h*3+j, free (pair, x) ->
                    # out[j, g*FM + pair*2F + h*F + x]
                    for h in range(2):
                        nc.gpsimd.dma_start(
                            out=bass.AP(tensor=out[:].tensor,
                                        offset=g * FM + h * F,
                                        ap=[[PIX, 3], [2 * F, NP], [1, F]]),
                            in_=y_sb[3 * h:3 * h + 3, :, :])

            if repeat == 1:
                body()
            else:
                with tc.For_i(0, repeat, 1):
                    body()

    nc.compile()
    return nc


_NC_CACHE: dict[int, object] = {}


def _get_nc(repeat: int = 1):
    if repeat not in _NC_CACHE:
        _NC_CACHE[repeat] = build_nc(repeat)
    return _NC_CACHE[repeat]


def make_in_maps(weight: np.ndarray, coor: np.ndarray) -> list[dict]:
    mats = _const_mats(coor)
    in_maps = []
    for k in range(N_CORES):
        b, hh = k // 2, k % 2
        shard = np.ascontiguousarray(
            weight[b, :, hh * 128:(hh + 1) * 128, :].reshape(N_CH, PIX),
            dtype=np.float16)
        in_maps.append({"w": shard, **mats})
    return in_maps


def assemble_out(results: list[dict]) -> np.ndarray:
    out = np.empty((B, 3, H, W), np.float32)
    for k in range(N_CORES):
        b, hh = k // 2, k % 2
        out[b, :, hh * 128:(hh + 1) * 128, :] = results[k]["out"].reshape(3, 128, W)
    return out


def kernel(input: np.ndarray, weight: np.ndarray, coor: np.ndarray) -> np.ndarray:
    nc = _get_nc(1)
    in_maps = make_in_maps(np.asarray(weight), np.asarray(coor))
    res = run_bass_kernel_spmd(nc, in_maps, core_ids=list(range(N_CORES)))
    return assemble_out(res.results)


# revision 33
# speedup vs baseline: 1.0776x; 1.0776x over previous
"""Trainium2 Bass kernel for the per-pixel locally-connected MLP (dense_mlp).

Reference computation (per batch b, pixel (h,w)):
    x0 = coor (2-vector, shared by all pixels)
    h1 = relu(W0 @ x0)        W0 = weight[b, 0:32].reshape(16, 2)   per pixel
    h2 = relu(W1 @ h1)        W1 = weight[b, 32:288].reshape(16,16) per pixel
    y  = W2 @ h2 + bias       W2 = weight[b, 288:336].reshape(3,16), bias = weight[b,336]
Output: [4, 3, 256, 256] float32.

Sharding: 8 cores, core k handles batch k//2, image rows (k%2)*128:(k%2+1)*128
=> per-core weight shard [337, 32768] (channels x pixels); no cross-core comm.

Implementation notes:
- Channels live on SBUF partitions, pixels on the free axis, so every weight
  load is a wide contiguous DMA; weights are cast to fp16 on the host to halve
  HBM traffic (the kernel is purely memory-bound; rel err ~5e-4).
- The per-pixel matvecs are elementwise multiplies (VectorE) plus
  partition-axis reductions (TensorE matmuls against small host-built 0/1
  selection matrices; `coor` is folded into the first matmul's stationary
  matrix, the bias channel into the last one's moving operand). Matmul
  operands use float32r (TF32-like) for 4x PE throughput over fp32.
- Sub-chunks of 512 px (one PSUM bank) are processed in pairs that share PSUM
  banks: the pair's two halves land at partitions 0:16 / 32:48 of one bank via
  zero-padded stationaries accumulating at a base-0 dst, so each ScalarE
  relu / output-copy covers two chunks in one op.
- Work is emitted stage-major over 4096-px macro-tiles (3 merged HWDGE loads
  per macro + SWDGE bias/output DMAs on gpsimd) so the in-order engine queues
  pipeline across sub-chunks; pool buffer counts are sized to keep 2-3 macros
  in flight without deadlocking the Tile scheduler.
"""

import sys

for _p in ("/opt/trn_rl_repo", "/root/.axon_site/_ro/trn_rl_repo"):
    if _p not in sys.path:
        sys.path.append(_p)

import numpy as np

import concourse.bass as bass
import concourse.tile as tile
from concourse import bacc, mybir
from concourse.bass_utils import run_bass_kernel_spmd

# ---------------------------------------------------------------- constants
B, H, W = 4, 256, 256
N_CH = 337            # 32 (L0) + 256 (L1) + 48 (L2) + 1 (bias)
N_CORES = 8
PIX = (B * H * W) // N_CORES  # 32768 pixels per core
F = 512               # pixels per compute chunk (one PSUM bank of fp32)
N_CHUNKS = PIX // F

FP32 = mybir.dt.float32
FP32R = mybir.dt.float32r
FP16 = mybir.dt.float16


def _const_mats(coor: np.ndarray) -> dict[str, np.ndarray]:
    """Small stationary matrices for the TensorE reductions."""
    cx, cy = float(coor[0]), float(coor[1])
    # S0 for a pair-stacked moving operand t0 [64, F]: rows 0:32 are the
    # even chunk's L0 weights -> cols 0:16, rows 32:64 the odd chunk's ->
    # cols 32:48. One matmul produces both halves of h1pre.
    s0 = np.zeros((64, 48), np.float32)
    for h in range(2):
        for i in range(16):
            s0[32 * h + 2 * i, 32 * h + i] = cx
            s0[32 * h + 2 * i + 1, 32 * h + i] = cy
    r8 = np.zeros((16, 128), np.float32)      # h1rep[m] = h1[m % 16]
    for m in range(128):
        r8[m % 16, m] = 1.0
    m1a = np.zeros((2, 128, 48), np.float32)  # h2pre[j] += sum_i prodA[16j+i]
    m1b = np.zeros((2, 128, 48), np.float32)
    for h in range(2):
        for k in range(128):
            m1a[h, k, 32 * h + k // 16] = 1.0
            m1b[h, k, 32 * h + 8 + k // 16] = 1.0
    # pair-fused layer-2: moving operand is h2pair [48,F] with chunk A's h2
    # at rows 0:16 and chunk B's at rows 32:48 (rows 16:32 are junk)
    r3_2 = np.zeros((48, 96), np.float32)     # h2rep2[k] = h2(k//48)[k % 16]
    for k in range(96):
        r3_2[(0 if k < 48 else 32) + k % 16, k] = 1.0
    m2b2 = np.zeros((98, 6), np.float32)      # y[h*3+j] = sum prodC + bias
    for k in range(96):
        m2b2[k, (k // 48) * 3 + (k % 48) // 16] = 1.0
    m2b2[96, 0:3] = 1.0                       # bias row, even chunk
    m2b2[97, 3:6] = 1.0                       # bias row, odd chunk
    return {"s0": s0.astype(np.float16), "r8": r8, "m1a": m1a, "m1b": m1b,
            "r3_2": r3_2, "m2b2": m2b2}


def build_nc(repeat: int = 1):
    """Build the per-core Bass program. `repeat` re-runs the whole kernel
    body sequentially (used only for differential HW timing)."""
    nc = bacc.Bacc(None, target_bir_lowering=False)

    w = nc.declare_dram_parameter("w", [N_CH, PIX], FP16, isOutput=False)
    out = nc.declare_dram_parameter("out", [3, PIX], FP32, isOutput=True)
    c_s0 = nc.declare_dram_parameter("s0", [64, 48], FP16, isOutput=False)
    c_r8 = nc.declare_dram_parameter("r8", [16, 128], FP32R, isOutput=False)
    c_m1a = nc.declare_dram_parameter("m1a", [2, 128, 48], FP32R, isOutput=False)
    c_m1b = nc.declare_dram_parameter("m1b", [2, 128, 48], FP32R, isOutput=False)
    c_r3_2 = nc.declare_dram_parameter("r3_2", [48, 96], FP32R, isOutput=False)
    c_m2b2 = nc.declare_dram_parameter("m2b2", [98, 6], FP32R, isOutput=False)

    G = 8                      # chunks per software-pipeline group
    with tile.TileContext(nc) as tc:
        with (
            tc.tile_pool(name="consts", bufs=1) as consts,
            tc.tile_pool(name="loads", bufs=2) as loads,
            tc.tile_pool(name="acts", bufs=4) as acts,
            tc.tile_pool(name="prods", bufs=3) as prods,
            tc.tile_pool(name="outs", bufs=2) as outs,
            tc.tile_pool(name="ps_sm16", bufs=2, space="PSUM") as ps_sm16,
            tc.tile_pool(name="ps_h2p", bufs=2, space="PSUM") as ps_h2p,
            tc.tile_pool(name="ps_rep", bufs=2, space="PSUM") as ps_rep
            ,tc.tile_pool(name="ps_y", bufs=2, space="PSUM") as ps_y,
        ):
            s0 = consts.tile([64, 48], FP16)
            r8 = consts.tile([48, 128], FP32R)   # rows 0:16 and 32:48 both
                                                 # hold R8 (for base 0/32)
            m1a = consts.tile([128, 2, 48], FP32R)
            m1b = consts.tile([128, 2, 48], FP32R)
            r3_2 = consts.tile([48, 96], FP32R)
            m2b2 = consts.tile([98, 6], FP32R)
            for t, d in ((s0, c_s0),
                         (r8[0:16, :], c_r8), (r8[32:48, :], c_r8),
                         (m1a, c_m1a.rearrange("h k m -> k h m")),
                         (m1b, c_m1b.rearrange("h k m -> k h m")),
                         (r3_2, c_r3_2), (m2b2, c_m2b2)):
                nc.sync.dma_start(out=t[:], in_=d[:])

            relu = mybir.ActivationFunctionType.Relu

            def body():
                # Macro-tile of G*F pixels; inside, sub-chunks are processed
                # in PAIRS sharing PSUM banks at partition offsets 0/32 (both
                # legal matmul base partitions). This halves ACT op count and
                # fuses all of layer 2 (rep, products, reduce+bias) per pair.
                FM = G * F
                NP = G // 2     # pairs per macro
                for g in range(N_CHUNKS // G):
                    mp = slice(g * FM, (g + 1) * FM)
                    sls = [slice(i * F, (i + 1) * F) for i in range(G)]
                    psl = [slice(p * F, (p + 1) * F) for p in range(NP)]

                    t0m = loads.tile([64, NP, F], FP16, tag="t0", name="t0m",
                                     bufs=3)
                    t1m = loads.tile([128, 2, FM], FP16, tag="t1", name="t1m", bufs=3)
                    # t2 pair-stacked: partition h*48+ch, free (pair, x);
                    # h = parity of the sub-chunk within its pair
                    t2m = loads.tile([96, NP, F], FP16, tag="t2", name="t2m", bufs=3)
                    pcm = prods.tile([98, NP, F], FP32R, tag="pcm", name="pcm",
                                     bufs=3)
                    for h in range(2):
                        nc.sync.dma_start(
                            out=t0m[32 * h:32 * h + 32, :, :],
                            in_=bass.AP(tensor=w[:].tensor,
                                        offset=g * FM + h * F,
                                        ap=[[PIX, 32], [2 * F, NP], [1, F]]))
                    nc.sync.dma_start(
                        out=t1m[:],
                        in_=w[32:288, mp].rearrange("(b p) x -> p b x", b=2))
                    for h in range(2):
                        nc.sync.dma_start(
                            out=t2m[48 * h:48 * h + 48, :, :],
                            in_=bass.AP(tensor=w[:].tensor,
                                        offset=288 * PIX + g * FM + h * F,
                                        ap=[[PIX, 48], [2 * F, NP], [1, F]]))
                        nc.gpsimd.dma_start(
                            out=pcm[96 + h:97 + h, :, :],
                            in_=bass.AP(tensor=w[:].tensor,
                                        offset=336 * PIX + g * FM + h * F,
                                        ap=[[2 * F, NP], [1, F]]))

                    h1pre = {}
                    for p in range(NP):
                        h1pre[p] = ps_sm16.tile([48, F], FP32, tag="sm16",
                                                name="h1pre")
                        nc.tensor.matmul(h1pre[p][:], s0[:],
                                         t0m[:, p, :],
                                         start=True, stop=True)
                    h1 = {}
                    for p in range(NP):
                        h1[p] = acts.tile([48, F], FP32R, tag="h1", name="h1")
                        nc.scalar.activation(h1[p][:], h1pre[p][:], relu)
                    h1rep = {}
                    for p in range(NP):
                        for h in range(2):
                            h1rep[p, h] = ps_rep.tile([128, F], FP32,
                                                      tag="rep", name="h1rep")
                            nc.tensor.matmul(
                                h1rep[p, h][:],
                                r8[32 * h:32 * h + 16, :],
                                h1[p][32 * h:32 * h + 16, :],
                                start=True, stop=True)
                    prodAB = {}
                    for p in range(NP):
                        for h in range(2):
                            prodAB[p, h] = prods.tile([128, 2, F], FP32R,
                                                      tag="prodAB",
                                                      name="prodAB", bufs=3)
                            rp = h1rep[p, h]
                            rep2 = bass.AP(tensor=rp.tensor,
                                           offset=rp[:].offset,
                                           ap=[rp[:].ap[0], [0, 2],
                                               rp[:].ap[1]])
                            nc.vector.tensor_mul(
                                prodAB[p, h][:],
                                t1m[:, :, sls[2 * p + h]], rep2)
                    h2pre = {}
                    for p in range(NP):
                        h2pre[p] = ps_h2p.tile([48, F], FP32, tag="h2p",
                                               name="h2pre")
                        for h in range(2):
                            nc.tensor.matmul(h2pre[p][:],
                                             m1a[:, h, :], prodAB[p, h][:, 0, :],
                                             start=(h == 0), stop=False)
                            nc.tensor.matmul(h2pre[p][:],
                                             m1b[:, h, :], prodAB[p, h][:, 1, :],
                                             start=False, stop=(h == 1))
                    h2 = {}
                    for p in range(NP):
                        h2[p] = acts.tile([48, F], FP32R, tag="h2", name="h2")
                        nc.scalar.activation(h2[p][:], h2pre[p][:], relu)
                    h2rep = {}
                    for p in range(NP):
                        h2rep[p] = ps_rep.tile([96, F], FP32, tag="rep",
                                               name="h2rep")
                        nc.tensor.matmul(h2rep[p][:], r3_2[:], h2[p][:],
                                         start=True, stop=True)
                    for p in range(NP):
                        nc.vector.tensor_mul(pcm[0:96, p, :],
                                             t2m[:, p, :], h2rep[p][:])
                    y = {}
                    for p in range(NP):
                        y[p] = ps_y.tile([6, F], FP32, tag="y", name="y")
                        nc.tensor.matmul(y[p][:], m2b2[:], pcm[:, p, :],
                                         start=True, stop=True)
                    y_sb = outs.tile([6, NP, F], FP32, tag="ysb", name="ysb",
                                     bufs=2)
                    for p in range(NP):
                        nc.scalar.copy(y_sb[:, p, :], y[p][:])
                    # y_sb partition h*3+j, free (pair, x) ->
                    # out[j, g*FM + pair*2F + h*F + x]
                    for h in range(2):
                        nc.gpsimd.dma_start(
                            out=bass.AP(tensor=out[:].tensor,
                                        offset=g * FM + h * F,
                                        ap=[[PIX, 3], [2 * F, NP], [1, F]]),
                            in_=y_sb[3 * h:3 * h + 3, :, :])

            if repeat == 1:
                body()
            else:
                with tc.For_i(0, repeat, 1):
                    body()

    nc.compile()
    return nc


_NC_CACHE: dict[int, object] = {}


def _get_nc(repeat: int = 1):
    if repeat not in _NC_CACHE:
        _NC_CACHE[repeat] = build_nc(repeat)
    return _NC_CACHE[repeat]


def make_in_maps(weight: np.ndarray, coor: np.ndarray) -> list[dict]:
    mats = _const_mats(coor)
    in_maps = []
    for k in range(N_CORES):
        b, hh = k // 2, k % 2
        shard = np.ascontiguousarray(
            weight[b, :, hh * 128:(hh + 1) * 128, :].reshape(N_CH, PIX),
            dtype=np.float16)
        in_maps.append({"w": shard, **mats})
    return in_maps


def assemble_out(results: list[dict]) -> np.ndarray:
    out = np.empty((B, 3, H, W), np.float32)
    for k in range(N_CORES):
        b, hh = k // 2, k % 2
        out[b, :, hh * 128:(hh + 1) * 128, :] = results[k]["out"].reshape(3, 128, W)
    return out


def kernel(input: np.ndarray, weight: np.ndarray, coor: np.ndarray) -> np.ndarray:
    nc = _get_nc(1)
    in_maps = make_in_maps(np.asarray(weight), np.asarray(coor))
    res = run_bass_kernel_spmd(nc, in_maps, core_ids=list(range(N_CORES)))
    return assemble_out(res.results)


# revision 37
# speedup vs baseline: 1.3012x; 1.2076x over previous
"""Trainium2 Bass kernel for the per-pixel locally-connected MLP (dense_mlp).

Reference computation (per batch b, pixel (h,w)):
    x0 = coor (2-vector, shared by all pixels)
    h1 = relu(W0 @ x0)        W0 = weight[b, 0:32].reshape(16, 2)   per pixel
    h2 = relu(W1 @ h1)        W1 = weight[b, 32:288].reshape(16,16) per pixel
    y  = W2 @ h2 + bias       W2 = weight[b, 288:336].reshape(3,16), bias = weight[b,336]
Output: [4, 3, 256, 256] float32.

Sharding: 8 cores, core k handles batch k//2, image rows (k%2)*128:(k%2+1)*128
=> per-core weight shard [337, 32768] (channels x pixels); no cross-core comm.

Implementation notes:
- Channels live on SBUF partitions, pixels on the free axis, so every weight
  load is a wide contiguous DMA; weights are cast to fp16 on the host to halve
  HBM traffic (the kernel is purely memory-bound; rel err ~5e-4).
- The per-pixel matvecs are elementwise multiplies (VectorE) plus
  partition-axis reductions (TensorE matmuls against small host-built 0/1
  selection matrices; `coor` is folded into the first matmul's stationary
  matrix, the bias channel into the last one's moving operand). Matmul
  operands use float32r (TF32-like) for 4x PE throughput over fp32.
- Sub-chunks of 512 px (one PSUM bank) are processed in pairs that share PSUM
  banks: the pair's two halves land at partitions 0:16 / 32:48 of one bank via
  zero-padded stationaries accumulating at a base-0 dst, so each ScalarE
  relu / output-copy covers two chunks in one op.
- Work is emitted stage-major over 4096-px macro-tiles (3 merged HWDGE loads
  per macro + SWDGE bias/output DMAs on gpsimd) so the in-order engine queues
  pipeline across sub-chunks; pool buffer counts are sized to keep 2-3 macros
  in flight without deadlocking the Tile scheduler.
"""

import sys

for _p in ("/opt/trn_rl_repo", "/root/.axon_site/_ro/trn_rl_repo"):
    if _p not in sys.path:
        sys.path.append(_p)

import numpy as np

import concourse.bass as bass
import concourse.tile as tile
from concourse import bacc, mybir
from concourse.bass_utils import run_bass_kernel_spmd

# ---------------------------------------------------------------- constants
B, H, W = 4, 256, 256
N_CH = 337            # 32 (L0) + 256 (L1) + 48 (L2) + 1 (bias)
N_CORES = 8
PIX = (B * H * W) // N_CORES  # 32768 pixels per core
F = 512               # pixels per compute chunk (one PSUM bank of fp32)
N_CHUNKS = PIX // F

FP32 = mybir.dt.float32
FP32R = mybir.dt.float32r
FP16 = mybir.dt.float16


def _const_mats(coor: np.ndarray) -> dict[str, np.ndarray]:
    """Small stationary matrices for the TensorE reductions."""
    cx, cy = float(coor[0]), float(coor[1])
    # S0 for a pair-stacked moving operand t0 [64, F]: rows 0:32 are the
    # even chunk's L0 weights -> cols 0:16, rows 32:64 the odd chunk's ->
    # cols 32:48. One matmul produces both halves of h1pre.
    s0 = np.zeros((64, 48), np.float32)
    for h in range(2):
        for i in range(16):
            s0[32 * h + 2 * i, 32 * h + i] = cx
            s0[32 * h + 2 * i + 1, 32 * h + i] = cy
    r8 = np.zeros((16, 128), np.float32)      # h1rep[m] = h1[m % 16]
    for m in range(128):
        r8[m % 16, m] = 1.0
    m1a = np.zeros((2, 128, 48), np.float32)  # h2pre[j] += sum_i prodA[16j+i]
    m1b = np.zeros((2, 128, 48), np.float32)
    for h in range(2):
        for k in range(128):
            m1a[h, k, 32 * h + k // 16] = 1.0
            m1b[h, k, 32 * h + 8 + k // 16] = 1.0
    # pair-fused layer-2: moving operand is h2pair [48,F] with chunk A's h2
    # at rows 0:16 and chunk B's at rows 32:48 (rows 16:32 are junk)
    r3_2 = np.zeros((48, 96), np.float32)     # h2rep2[k] = h2(k//48)[k % 16]
    for k in range(96):
        r3_2[(0 if k < 48 else 32) + k % 16, k] = 1.0
    m2b2 = np.zeros((98, 6), np.float32)      # y[h*3+j] = sum prodC + bias
    for k in range(96):
        m2b2[k, (k // 48) * 3 + (k % 48) // 16] = 1.0
    m2b2[96, 0:3] = 1.0                       # bias row, even chunk
    m2b2[97, 3:6] = 1.0                       # bias row, odd chunk
    return {"s0": s0.astype(np.float16), "r8": r8, "m1a": m1a, "m1b": m1b,
            "r3_2": r3_2, "m2b2": m2b2}


def build_nc(repeat: int = 1):
    """Build the per-core Bass program. `repeat` re-runs the whole kernel
    body sequentially (used only for differential HW timing)."""
    nc = bacc.Bacc(None, target_bir_lowering=False)

    w = nc.declare_dram_parameter("w", [N_CH, PIX], FP16, isOutput=False)
    wb = nc.declare_dram_parameter("wb", [1, PIX], FP32R, isOutput=False)
    out = nc.declare_dram_parameter("out", [3, PIX], FP32, isOutput=True)
    c_s0 = nc.declare_dram_parameter("s0", [64, 48], FP16, isOutput=False)
    c_r8 = nc.declare_dram_parameter("r8", [16, 128], FP32R, isOutput=False)
    c_m1a = nc.declare_dram_parameter("m1a", [2, 128, 48], FP32R, isOutput=False)
    c_m1b = nc.declare_dram_parameter("m1b", [2, 128, 48], FP32R, isOutput=False)
    c_r3_2 = nc.declare_dram_parameter("r3_2", [48, 96], FP32R, isOutput=False)
    c_m2b2 = nc.declare_dram_parameter("m2b2", [98, 6], FP32R, isOutput=False)

    G = 8                      # chunks per software-pipeline group
    with tile.TileContext(nc) as tc:
        with (
            tc.tile_pool(name="consts", bufs=1) as consts,
            tc.tile_pool(name="loads", bufs=2) as loads,
            tc.tile_pool(name="acts", bufs=4) as acts,
            tc.tile_pool(name="prods", bufs=3) as prods,
            tc.tile_pool(name="outs", bufs=2) as outs,
            tc.tile_pool(name="ps_sm16", bufs=2, space="PSUM") as ps_sm16,
            tc.tile_pool(name="ps_h2p", bufs=2, space="PSUM") as ps_h2p,
            tc.tile_pool(name="ps_rep", bufs=2, space="PSUM") as ps_rep
            ,tc.tile_pool(name="ps_y", bufs=2, space="PSUM") as ps_y,
        ):
            s0 = consts.tile([64, 48], FP16)
            r8 = consts.tile([48, 128], FP32R)   # rows 0:16 and 32:48 both
                                                 # hold R8 (for base 0/32)
            m1a = consts.tile([128, 2, 48], FP32R)
            m1b = consts.tile([128, 2, 48], FP32R)
            r3_2 = consts.tile([48, 96], FP32R)
            m2b2 = consts.tile([98, 6], FP32R)
            for t, d in ((s0, c_s0),
                         (r8[0:16, :], c_r8), (r8[32:48, :], c_r8),
                         (m1a, c_m1a.rearrange("h k m -> k h m")),
                         (m1b, c_m1b.rearrange("h k m -> k h m")),
                         (r3_2, c_r3_2), (m2b2, c_m2b2)):
                nc.sync.dma_start(out=t[:], in_=d[:])

            relu = mybir.ActivationFunctionType.Relu

            def body():
                # Macro-tile of G*F pixels; inside, sub-chunks are processed
                # in PAIRS sharing PSUM banks at partition offsets 0/32 (both
                # legal matmul base partitions). This halves ACT op count and
                # fuses all of layer 2 (rep, products, reduce+bias) per pair.
                FM = G * F
                NP = G // 2     # pairs per macro
                for g in range(N_CHUNKS // G):
                    mp = slice(g * FM, (g + 1) * FM)
                    sls = [slice(i * F, (i + 1) * F) for i in range(G)]
                    psl = [slice(p * F, (p + 1) * F) for p in range(NP)]

                    t0m = loads.tile([64, NP, F], FP16, tag="t0", name="t0m",
                                     bufs=3)
                    t1m = loads.tile([128, 2, FM], FP16, tag="t1", name="t1m", bufs=3)
                    # t2 pair-stacked: partition h*48+ch, free (pair, x);
                    # h = parity of the sub-chunk within its pair
                    t2m = loads.tile([96, NP, F], FP16, tag="t2", name="t2m", bufs=3)
                    pcm = prods.tile([98, NP, F], FP32R, tag="pcm", name="pcm",
                                     bufs=3)
                    for h in range(2):
                        nc.sync.dma_start(
                            out=t0m[32 * h:32 * h + 32, :, :],
                            in_=bass.AP(tensor=w[:].tensor,
                                        offset=g * FM + h * F,
                                        ap=[[PIX, 32], [2 * F, NP], [1, F]]))
                    nc.sync.dma_start(
                        out=t1m[:],
                        in_=w[32:288, mp].rearrange("(b p) x -> p b x", b=2))
                    for h in range(2):
                        nc.sync.dma_start(
                            out=t2m[48 * h:48 * h + 48, :, :],
                            in_=bass.AP(tensor=w[:].tensor,
                                        offset=288 * PIX + g * FM + h * F,
                                        ap=[[PIX, 48], [2 * F, NP], [1, F]]))
                        nc.sync.dma_start(
                            out=pcm[96 + h:97 + h, :, :],
                            in_=bass.AP(tensor=wb[:].tensor,
                                        offset=g * FM + h * F,
                                        ap=[[2 * F, NP], [1, F]]))

                    h1pre = {}
                    for p in range(NP):
                        h1pre[p] = ps_sm16.tile([48, F], FP32, tag="sm16",
                                                name="h1pre")
                        nc.tensor.matmul(h1pre[p][:], s0[:],
                                         t0m[:, p, :],
                                         start=True, stop=True)
                    h1 = {}
                    for p in range(NP):
                        h1[p] = acts.tile([48, F], FP32R, tag="h1", name="h1")
                        nc.scalar.activation(h1[p][:], h1pre[p][:], relu)
                    h1rep = {}
                    for p in range(NP):
                        for h in range(2):
                            h1rep[p, h] = ps_rep.tile([128, F], FP32,
                                                      tag="rep", name="h1rep")
                            nc.tensor.matmul(
                                h1rep[p, h][:],
                                r8[32 * h:32 * h + 16, :],
                                h1[p][32 * h:32 * h + 16, :],
                                start=True, stop=True)
                    prodAB = {}
                    for p in range(NP):
                        for h in range(2):
                            prodAB[p, h] = prods.tile([128, 2, F], FP32R,
                                                      tag="prodAB",
                                                      name="prodAB", bufs=3)
                            rp = h1rep[p, h]
                            rep2 = bass.AP(tensor=rp.tensor,
                                           offset=rp[:].offset,
                                           ap=[rp[:].ap[0], [0, 2],
                                               rp[:].ap[1]])
                            nc.vector.tensor_mul(
                                prodAB[p, h][:],
                                t1m[:, :, sls[2 * p + h]], rep2)
                    h2pre = {}
                    for p in range(NP):
                        h2pre[p] = ps_h2p.tile([48, F], FP32, tag="h2p",
                                               name="h2pre")
                        for h in range(2):
                            nc.tensor.matmul(h2pre[p][:],
                                             m1a[:, h, :], prodAB[p, h][:, 0, :],
                                             start=(h == 0), stop=False)
                            nc.tensor.matmul(h2pre[p][:],
                                             m1b[:, h, :], prodAB[p, h][:, 1, :],
                                             start=False, stop=(h == 1))
                    h2 = {}
                    for p in range(NP):
                        h2[p] = acts.tile([48, F], FP32R, tag="h2", name="h2")
                        nc.scalar.activation(h2[p][:], h2pre[p][:], relu)
                    h2rep = {}
                    for p in range(NP):
                        h2rep[p] = ps_rep.tile([96, F], FP32, tag="rep",
                                               name="h2rep")
                        nc.tensor.matmul(h2rep[p][:], r3_2[:], h2[p][:],
                                         start=True, stop=True)
                    for p in range(NP):
                        nc.vector.tensor_mul(pcm[0:96, p, :],
                                             t2m[:, p, :], h2rep[p][:])
                    y = {}
                    for p in range(NP):
                        y[p] = ps_y.tile([6, F], FP32, tag="y", name="y")
                        nc.tensor.matmul(y[p][:], m2b2[:], pcm[:, p, :],
                                         start=True, stop=True)
                    y_sb = outs.tile([6, NP, F], FP32, tag="ysb", name="ysb",
                                     bufs=2)
                    for p in range(NP):
                        nc.scalar.copy(y_sb[:, p, :], y[p][:])
                    # y_sb partition h*3+j, free (pair, x) ->
                    # out[j, g*FM + pair*2F + h*F + x]
                    for h in range(2):
                        nc.sync.dma_start(
                            out=bass.AP(tensor=out[:].tensor,
                                        offset=g * FM + h * F,
                                        ap=[[PIX, 3], [2 * F, NP], [1, F]]),
                            in_=y_sb[3 * h:3 * h + 3, :, :])

            if repeat == 1:
                body()
            else:
                with tc.For_i(0, repeat, 1):
                    body()

    nc.compile()
    return nc


_NC_CACHE: dict[int, object] = {}


def _get_nc(repeat: int = 1):
    if repeat not in _NC_CACHE:
        _NC_CACHE[repeat] = build_nc(repeat)
    return _NC_CACHE[repeat]


def make_in_maps(weight: np.ndarray, coor: np.ndarray) -> list[dict]:
    mats = _const_mats(coor)
    in_maps = []
    for k in range(N_CORES):
        b, hh = k // 2, k % 2
        shard = np.ascontiguousarray(
            weight[b, :, hh * 128:(hh + 1) * 128, :].reshape(N_CH, PIX),
            dtype=np.float16)
        in_maps.append({"w": shard,
                        "wb": weight[b, 336, hh * 128:(hh + 1) * 128, :]
                        .reshape(1, PIX).astype(np.float32), **mats})
    return in_maps


def assemble_out(results: list[dict]) -> np.ndarray:
    out = np.empty((B, 3, H, W), np.float32)
    for k in range(N_CORES):
        b, hh = k // 2, k % 2
        out[b, :, hh * 128:(hh + 1) * 128, :] = results[k]["out"].reshape(3, 128, W)
    return out


def kernel(input: np.ndarray, weight: np.ndarray, coor: np.ndarray) -> np.ndarray:
    nc = _get_nc(1)
    in_maps = make_in_maps(np.asarray(weight), np.asarray(coor))
    res = run_bass_kernel_spmd(nc, in_maps, core_ids=list(range(N_CORES)))
    return assemble_out(res.results)
